# revision 7
# baseline (speedup 1.0000x reference)
"""Compressible Ogden strain-energy kernel for Trainium2 (Bass/Tile), 8-core SPMD.

W(F) per quadrature point:
  C  = F^T F                          (symmetric 3x3)
  J  = sqrt(det C)
  Cb = J^(-2/3) C,  lamb = eigvals(Cb)
  W  = sum_k mu_k/alpha_k (sum_i lamb_i^(alpha_k/2) - 3)
     + KAPPA/BETA^2 (J^BETA - BETA ln J - 1)

Device-side math (all elementwise over [128, T] fp32 planes):
  - C entries and invariants q = tr(C)/3, p2 = tr((C-qI)^2), ds = det(C-qI)
  - det C = q^3 + ds - q p2/2  (char. poly identity, avoids a 2nd det expansion)
  - eig(C): trigonometric Cardano.  r = ds / (2 p^3), p = sqrt(p2/6),
    u = acos(r) built from Arctan:  u = pi/2 - arctan(r/sqrt(1-r^2)),
    cos terms via Sin activation:   cos(u/3) = sin(2pi/3 - arctan/3) etc.
  - eig(Cb) = detC^(-1/3) eig(C) folded in log space:  ln lamb = ln lamC - t/3
  - powers via Exp(alpha_k/2 * ln lamb + ln|mu_k/alpha_k|)
  - W_vol = K/B^2 (detC - t - 1) with t = ln detC   (BETA=2 exact identity)

Activation table sets: everything lives in natural_log_exp_and_others except
arctan/sin (trig_and_small) -> phase-ordered emission keeps set switches at 2.
"""

import math

import numpy as np

import concourse.bacc as bacc
import concourse.mybir as mybir
import concourse.tile as tile
from concourse.bass_utils import run_bass_kernel_spmd

P = 128
NCORES = 8
KAPPA = 100.0
BETA = 2.0
F32 = mybir.dt.float32
AF = mybir.ActivationFunctionType
OP = mybir.AluOpType

RCLAMP = 1.0 - 1e-6
V_EPS = 1e-12
PI = math.pi


class Planes:
    """Contiguous-run plane allocator inside one big [P, NP*T] SBUF tile."""

    def __init__(self, ws, T, n):
        self.ws = ws
        self.T = T
        self.free_set = set(range(n))
        self.peak = 0
        self.n = n

    def alloc(self, k=1):
        free = sorted(self.free_set)
        run = None
        for i in range(len(free) - k + 1):
            if free[i + k - 1] - free[i] == k - 1:
                run = free[i]
                break
        if run is None:
            raise RuntimeError(f"no {k} contiguous planes free (free={free})")
        for j in range(run, run + k):
            self.free_set.remove(j)
        self.peak = max(self.peak, self.n - len(self.free_set))
        return run

    def release(self, base, k=1):
        for j in range(base, base + k):
            assert j not in self.free_set
            self.free_set.add(j)

    def ap(self, base, k=1):
        T = self.T
        return self.ws[:, base * T:(base + k) * T]


def _phase_a(nc, pl, st, Fm, ch, T):
    """Input DMA through g = rc/sqrt(1-rc^2).  ACT funcs: Square/Ln/Exp/Copy."""
    ft = pl.alloc(9)
    nc.sync.dma_start(out=pl.ap(ft, 9), in_=Fm[:, ch * 9 * T:(ch + 1) * 9 * T])

    # squares of all 9 F entries, deinterleaved (t c) -> planes (c t)
    sq = pl.alloc(9)
    ft_ct = pl.ap(ft, 9).rearrange("p (t c) -> p c t", c=9)
    sq_ct = pl.ap(sq, 9).rearrange("p (c t) -> p c t", t=T)
    nc.scalar.activation(sq_ct, ft_ct, AF.Square)

    # off-diagonal products: cXY_i = F[i,x]*F[i,y]; F col x = comps {x,x+3,x+6}
    pr = pl.alloc(9)
    ftg = pl.ap(ft, 9).rearrange("p (t a b) -> p b a t", a=3, b=3)
    prg = pl.ap(pr, 9).rearrange("p (g i t) -> p g i t", g=3, i=3)
    nc.vector.tensor_mul(prg[:, 0], ftg[:, 0], ftg[:, 1])   # c01 terms
    nc.vector.tensor_mul(prg[:, 1], ftg[:, 0], ftg[:, 2])   # c02 terms
    nc.vector.tensor_mul(prg[:, 2], ftg[:, 1], ftg[:, 2])   # c12 terms
    pl.release(ft, 9)

    # diag C = [c00 c11 c22]: sum of squares blocks
    cd = pl.alloc(3)
    nc.vector.tensor_add(pl.ap(cd, 3), pl.ap(sq, 3), pl.ap(sq + 3, 3))
    nc.vector.tensor_add(pl.ap(cd, 3), pl.ap(cd, 3), pl.ap(sq + 6, 3))
    pl.release(sq, 9)

    # offdiag C = [c01 c02 c12]: sum the 3 terms of each group
    co = pl.alloc(3)
    prt = pl.ap(pr, 9).rearrange("p (g i t) -> p i g t", g=3, i=3)
    co3 = pl.ap(co, 3).rearrange("p (g t) -> p g t", g=3)
    nc.vector.tensor_add(co3, prt[:, 0], prt[:, 1])
    nc.vector.tensor_add(co3, co3, prt[:, 2])
    pl.release(pr, 9)

    # q = tr(C)/3 ; D = C_diag - q
    t1 = pl.alloc(1)
    nc.vector.tensor_add(pl.ap(t1), pl.ap(cd), pl.ap(cd + 1))
    nc.vector.tensor_add(pl.ap(t1), pl.ap(t1), pl.ap(cd + 2))
    q = pl.alloc(1)
    nc.scalar.activation(pl.ap(q), pl.ap(t1), AF.Copy, scale=1.0 / 3.0)
    dd = pl.alloc(3)
    t1b = pl.ap(t1).unsqueeze(1).broadcast_to([P, 3, T])
    cd3 = pl.ap(cd, 3).rearrange("p (i t) -> p i t", i=3)
    dd3 = pl.ap(dd, 3).rearrange("p (i t) -> p i t", i=3)
    nc.vector.scalar_tensor_tensor(dd3, t1b, -1.0 / 3.0, cd3, OP.mult, OP.add)
    pl.release(t1)
    pl.release(cd, 3)

    # squares of offdiag and deviatoric diag
    osq = pl.alloc(3)
    nc.scalar.activation(pl.ap(osq, 3), pl.ap(co, 3), AF.Square)
    dsq = pl.alloc(3)
    nc.scalar.activation(pl.ap(dsq, 3), pl.ap(dd, 3), AF.Square)

    # p2 = sum(dsq) + 2*sum(osq); v = p2/6 + eps
    p1 = pl.alloc(1)
    nc.vector.tensor_add(pl.ap(p1), pl.ap(osq), pl.ap(osq + 1))
    nc.vector.tensor_add(pl.ap(p1), pl.ap(p1), pl.ap(osq + 2))
    sd = pl.alloc(1)
    nc.vector.tensor_add(pl.ap(sd), pl.ap(dsq), pl.ap(dsq + 1))
    nc.vector.tensor_add(pl.ap(sd), pl.ap(sd), pl.ap(dsq + 2))
    pl.release(dsq, 3)
    p2 = pl.alloc(1)
    nc.vector.scalar_tensor_tensor(pl.ap(p2), pl.ap(p1), 2.0, pl.ap(sd), OP.mult, OP.add)
    pl.release(p1)
    pl.release(sd)
    v = pl.alloc(1)
    nc.vector.tensor_scalar(pl.ap(v), pl.ap(p2), 1.0 / 6.0, V_EPS, OP.mult, OP.add)

    # lnv; p = v^0.5; w = 0.5 v^-1.5
    lnv = pl.alloc(1)
    nc.scalar.activation(pl.ap(lnv), pl.ap(v), AF.Ln)
    pl.release(v)
    pp = pl.alloc(1)
    nc.scalar.activation(pl.ap(pp), pl.ap(lnv), AF.Exp, scale=0.5)
    w = pl.alloc(1)
    nc.scalar.activation(pl.ap(w), pl.ap(lnv), AF.Exp, scale=-1.5, bias=math.log(0.5))
    pl.release(lnv)

    # ds = det(C - qI) via cofactor expansion (reusing osq for c12^2)
    m1 = pl.alloc(1)
    nc.vector.tensor_mul(pl.ap(m1), pl.ap(dd + 1), pl.ap(dd + 2))
    nc.vector.tensor_sub(pl.ap(m1), pl.ap(m1), pl.ap(osq + 2))          # y1
    m2 = pl.alloc(1)
    nc.vector.tensor_mul(pl.ap(m2), pl.ap(co), pl.ap(dd + 2))
    m3 = pl.alloc(1)
    nc.vector.tensor_mul(pl.ap(m3), pl.ap(co + 1), pl.ap(co + 2))
    nc.vector.tensor_sub(pl.ap(m2), pl.ap(m2), pl.ap(m3))               # y2
    m4 = pl.alloc(1)
    nc.vector.tensor_mul(pl.ap(m4), pl.ap(co), pl.ap(co + 2))
    nc.vector.tensor_mul(pl.ap(m3), pl.ap(co + 1), pl.ap(dd + 1))
    nc.vector.tensor_sub(pl.ap(m4), pl.ap(m4), pl.ap(m3))               # y3
    pl.release(m3)
    pl.release(osq, 3)
    nc.vector.tensor_mul(pl.ap(m1), pl.ap(dd), pl.ap(m1))               # d0*y1
    nc.vector.tensor_mul(pl.ap(m2), pl.ap(co), pl.ap(m2))               # c01*y2
    nc.vector.tensor_mul(pl.ap(m4), pl.ap(co + 1), pl.ap(m4))           # c02*y3
    pl.release(co, 3)
    pl.release(dd, 3)
    nc.vector.tensor_sub(pl.ap(m1), pl.ap(m1), pl.ap(m2))
    nc.vector.tensor_add(pl.ap(m1), pl.ap(m1), pl.ap(m4))               # ds
    pl.release(m2)
    pl.release(m4)
    ds = m1

    # detC = q^3 + ds - 0.5 q p2 ; t = ln detC
    qsq = pl.alloc(1)
    nc.scalar.activation(pl.ap(qsq), pl.ap(q), AF.Square)
    nc.vector.tensor_mul(pl.ap(qsq), pl.ap(qsq), pl.ap(q))              # q^3
    qp2 = pl.alloc(1)
    nc.vector.tensor_mul(pl.ap(qp2), pl.ap(q), pl.ap(p2))
    pl.release(p2)
    nc.vector.tensor_add(pl.ap(qsq), pl.ap(qsq), pl.ap(ds))
    detc = pl.alloc(1)
    nc.vector.scalar_tensor_tensor(pl.ap(detc), pl.ap(qp2), -0.5, pl.ap(qsq), OP.mult, OP.add)
    pl.release(qsq)
    pl.release(qp2)
    tt = pl.alloc(1)
    nc.scalar.activation(pl.ap(tt), pl.ap(detc), AF.Ln)

    # r = ds * w, clamped.  Quarter-angle form keeps the arctan arg in [0,1]:
    #   h2 = cos(u/2) = sqrt((1+r)/2);  tan(u/4) = sqrt((1-h2)/(1+h2))
    nc.vector.tensor_mul(pl.ap(ds), pl.ap(ds), pl.ap(w))
    pl.release(w)
    rc = ds
    nc.vector.tensor_scalar(pl.ap(rc), pl.ap(rc), -RCLAMP, RCLAMP, OP.max, OP.min)
    aa = pl.alloc(1)
    nc.vector.tensor_scalar(pl.ap(aa), pl.ap(rc), 0.5, 0.5, OP.mult, OP.add)
    pl.release(rc)
    nc.scalar.activation(pl.ap(aa), pl.ap(aa), AF.Ln)
    h2 = pl.alloc(1)
    nc.scalar.activation(pl.ap(h2), pl.ap(aa), AF.Exp, scale=0.5)
    pl.release(aa)
    hm = pl.alloc(1)
    nc.vector.tensor_scalar(pl.ap(hm), pl.ap(h2), -1.0, 1.0, OP.mult, OP.add)
    nc.vector.tensor_scalar(pl.ap(h2), pl.ap(h2), 1.0, None, OP.add)    # 1+h2
    nc.scalar.activation(pl.ap(hm), pl.ap(hm), AF.Ln)
    nc.scalar.activation(pl.ap(h2), pl.ap(h2), AF.Ln)
    nc.vector.tensor_sub(pl.ap(hm), pl.ap(hm), pl.ap(h2))
    pl.release(h2)
    nc.scalar.activation(pl.ap(hm), pl.ap(hm), AF.Exp, scale=0.5)       # tan(u/4)

    st.update(g=hm, q=q, p=pp, detc=detc, t=tt)


def _phase_b(nc, pl, st, T):
    """Trig set: ar = arctan(g); lamC via cos terms; then lnlam (set A again
    only in phase C -- the two Sin calls and Arctan share trig_and_small)."""
    g = st.pop("g")
    nc.scalar.activation(pl.ap(g), pl.ap(g), AF.Arctan)                  # u/4
    ar = g
    c1 = pl.alloc(1)
    nc.scalar.activation(pl.ap(c1), pl.ap(ar), AF.Sin, scale=4.0 / 3.0, bias=PI / 2.0)
    c2 = pl.alloc(1)
    nc.scalar.activation(pl.ap(c2), pl.ap(ar), AF.Sin, scale=4.0 / 3.0, bias=-5.0 * PI / 6.0)
    pl.release(ar)

    q, pp = st.pop("q"), st.pop("p")
    lam = pl.alloc(3)
    nc.vector.scalar_tensor_tensor(pl.ap(lam), pl.ap(c1), 2.0, pl.ap(pp), OP.mult, OP.mult)
    nc.vector.tensor_add(pl.ap(lam), pl.ap(lam), pl.ap(q))
    nc.vector.scalar_tensor_tensor(pl.ap(lam + 2), pl.ap(c2), 2.0, pl.ap(pp), OP.mult, OP.mult)
    nc.vector.tensor_add(pl.ap(lam + 2), pl.ap(lam + 2), pl.ap(q))
    nc.vector.scalar_tensor_tensor(pl.ap(lam + 1), pl.ap(q), 3.0, pl.ap(lam), OP.mult, OP.subtract)
    nc.vector.tensor_sub(pl.ap(lam + 1), pl.ap(lam + 1), pl.ap(lam + 2))
    pl.release(c1)
    pl.release(c2)
    pl.release(q)
    pl.release(pp)
    st.update(lam=lam)


def _phase_c(nc, pl, st, Wm, ch, T, alp2, lncoef, sgn, k0):
    """Set A again: ln lam, exponent sums, volumetric part, output DMA."""
    lam = st.pop("lam")
    detc, tt = st.pop("detc"), st.pop("t")
    nc.scalar.activation(pl.ap(lam, 3), pl.ap(lam, 3), AF.Ln)
    lnl = lam
    ttb = pl.ap(tt).unsqueeze(1).broadcast_to([P, 3, T])
    lnl3 = pl.ap(lnl, 3).rearrange("p (i t) -> p i t", i=3)
    nc.vector.scalar_tensor_tensor(lnl3, ttb, -1.0 / 3.0, lnl3, OP.mult, OP.add)

    ee = pl.alloc(9)
    live_k = [k for k in range(3) if lncoef[k] is not None]
    for k in live_k:
        nc.scalar.activation(pl.ap(ee + 3 * k, 3), pl.ap(lnl, 3), AF.Exp,
                             scale=float(alp2[k]), bias=float(lncoef[k]))
    pl.release(lnl, 3)

    # pw_k = sum_i e_ik  (grouped over k), then signed sum + constants
    pw = pl.alloc(3)
    eg = pl.ap(ee, 9).rearrange("p (k i t) -> p i k t", k=3, i=3)
    pw3 = pl.ap(pw, 3).rearrange("p (k t) -> p k t", k=3)
    nc.vector.tensor_add(pw3, eg[:, 0], eg[:, 1])
    nc.vector.tensor_add(pw3, pw3, eg[:, 2])
    pl.release(ee, 9)
    for k in live_k:
        if sgn[k] < 0:
            nc.vector.tensor_scalar(pl.ap(pw + k), pl.ap(pw + k), -1.0, None, OP.mult)

    acc = pl.alloc(1)
    if not live_k:
        nc.vector.memset(pl.ap(acc), 0.0)
    else:
        nc.vector.tensor_copy(pl.ap(acc), pl.ap(pw + live_k[0]))
        for k in live_k[1:]:
            nc.vector.tensor_add(pl.ap(acc), pl.ap(acc), pl.ap(pw + k))
    pl.release(pw, 3)

    # W = acc + K/B^2 * (detC - t) + k0
    nc.vector.tensor_sub(pl.ap(detc), pl.ap(detc), pl.ap(tt))
    pl.release(tt)
    wout = pl.alloc(1)
    nc.vector.scalar_tensor_tensor(pl.ap(wout), pl.ap(detc), KAPPA / (BETA * BETA),
                                   pl.ap(acc), OP.mult, OP.add)
    pl.release(detc)
    pl.release(acc)
    nc.vector.tensor_scalar(pl.ap(wout), pl.ap(wout), float(k0), None, OP.add)
    nc.sync.dma_start(out=Wm[:, ch * T:(ch + 1) * T], in_=pl.ap(wout))
    pl.release(wout)


def build_nc(T, chunks, mu, alpha, debug=False, nplanes=40):
    """Build the SPMD single-core program (same program runs on all cores)."""
    mu64 = np.asarray(mu, np.float64)
    al64 = np.asarray(alpha, np.float64)
    alp2 = al64 * 0.5
    coef = mu64 / al64
    lncoef = [None if c == 0.0 else math.log(abs(c)) for c in coef]
    sgn = [0.0 if c == 0.0 else math.copysign(1.0, c) for c in coef]
    k0 = -KAPPA / (BETA * BETA) - 3.0 * float(np.sum(coef))

    nc = bacc.Bacc("TRN2", target_bir_lowering=False, debug=debug)

    # Register the non-zero activation bias constants (same preamble pattern
    # Bass.__init__ uses for 0.0/1.0): [128,1] SBUF tensors memset once.
    bias_vals = {math.log(0.5), PI / 2.0, -5.0 * PI / 6.0}
    bias_vals.update(float(b) for b in lncoef if b is not None)
    for val in sorted(bias_vals):
        if (F32, val) in nc.const_aps.aps:
            continue
        tns = nc.alloc_sbuf_tensor(f"const-f32-{val!r}", [128, 1], F32)
        nc.gpsimd.memset(tns.ap(), val)
        nc.const_aps.aps[(F32, val)] = tns.ap()
    nc.all_engine_barrier()

    TC = T * chunks
    Fm = nc.dram_tensor("F", [P, 9 * TC], F32, kind="ExternalInput")
    Wm = nc.dram_tensor("W", [P, TC], F32, kind="ExternalOutput")

    with tile.TileContext(nc) as tc:
        with tc.tile_pool(name="ws", bufs=1) as pool:
            pls, sts = [], []
            for ch in range(chunks):
                ws = pool.tile([P, nplanes * T], F32, tag=f"ws{ch}")
                pls.append(Planes(ws, T, nplanes))
                sts.append({})
            for ch in range(chunks):
                _phase_a(nc, pls[ch], sts[ch], Fm, ch, T)
            for ch in range(chunks):
                _phase_b(nc, pls[ch], sts[ch], T)
            for ch in range(chunks):
                _phase_c(nc, pls[ch], sts[ch], Wm, ch, T, alp2, lncoef, sgn, k0)
    nc.compile()
    return nc


def _pad_and_shard(F, T, chunks):
    n = F.shape[0]
    per_core = P * T * chunks
    npad = NCORES * per_core
    flat = np.ascontiguousarray(F, dtype=np.float32).reshape(n, 9)
    if npad > n:
        pad = np.tile(np.eye(3, dtype=np.float32).reshape(1, 9), (npad - n, 1))
        flat = np.concatenate([flat, pad], axis=0)
    return flat.reshape(NCORES, P, 9 * T * chunks)


def kernel(F, mu, alpha):
    F = np.asarray(F)
    n = F.shape[0]
    chunks = 2
    # per-chunk plane length: even (DVE 2x tensor_scalar mode), cover n
    T = -(-n // (NCORES * P * chunks))
    T += T % 2
    shards = _pad_and_shard(F, T, chunks)
    nc = build_nc(T, chunks, mu, alpha)
    in_maps = [{"F": shards[i]} for i in range(NCORES)]
    res = run_bass_kernel_spmd(nc, in_maps, list(range(NCORES)))
    out = np.concatenate([res.results[i]["W"].reshape(-1) for i in range(NCORES)])
    return out[:n].astype(np.float32, copy=False)


if __name__ == "__main__":
    rng = np.random.default_rng(0)
    F = np.eye(3, dtype=np.float32) + 0.1 * rng.standard_normal((4096, 3, 3), dtype=np.float32)
    mu = np.array([0.63, 0.0012, -0.01], np.float32)
    alpha = np.array([1.3, 5.0, -2.0], np.float32)
    print(kernel(F, mu, alpha)[:8])
